# revision 1
# baseline (speedup 1.0000x reference)
"""Causal self-attention Trainium2 kernel.

Full computation: y = softmax_causal((x@Wq)(x@Wk)^T / sqrt(D)) @ (x@Wv) @ Wp
Sharding: head-parallel over 8 cores (H=8 heads, one per core), both batches
on every core (batch 0 on SBUF partitions 0:64, batch 1 on 64:128).
Each core produces a partial output (its head's contribution to y @ W_proj);
the host sums the 8 partials.
"""

import sys

sys.path.insert(0, "/opt/trn_rl_repo")

from contextlib import ExitStack

import numpy as np

import concourse.bass as bass
import concourse.mybir as mybir
import concourse.tile as tile
from concourse import bacc

B, T, C, H, D = 2, 4096, 512, 8, 64
BT = B * T  # 8192
NCORES = 8
NC_CH = C // 128  # 4 contraction chunks for the QKV projection
NQT = T // 512  # 8 q-tiles per batch
NKT = T // 128  # 32 k-tiles per batch
KGRP = 3  # k-tiles per exp group (3 PSUM banks, double buffered)

f32 = mybir.dt.float32
f32r = mybir.dt.float32r
bf16 = mybir.dt.bfloat16


def _r(ap):
    return ap  # tiles are fp32r-typed now


def build_kernel() -> bass.Bass:
    nc = bacc.Bacc()

    xT = nc.dram_tensor("xT", [C, BT], bf16, kind="ExternalInput")
    wq = nc.dram_tensor("wq", [C, D], bf16, kind="ExternalInput")
    wk = nc.dram_tensor("wk", [C, D], bf16, kind="ExternalInput")
    wv = nc.dram_tensor("wv", [C, D], bf16, kind="ExternalInput")
    # wp row D is zeros; rows 0:D are this head's W_proj slice.
    wp = nc.dram_tensor("wp", [D + 1, C], f32r, kind="ExternalInput")
    ev = nc.dram_tensor("ev", [D + 1, 2], f32r, kind="ExternalInput")
    ones64 = nc.dram_tensor("ones64", [64], f32r, kind="ExternalInput")
    outp = nc.dram_tensor("outp", [BT, C], f32, kind="ExternalOutput")

    xTr = xT[:, :].rearrange("(a p) t -> a p t", p=128)  # [4, 128, BT]

    with tile.TileContext(nc) as tc, ExitStack() as ctx:
        singles = ctx.enter_context(tc.tile_pool(name="singles", bufs=1))

        # Persistent SBUF tensors
        qT = singles.tile([128, T], f32r)  # [0:64]=batch0 head dims, [64:128]=batch1
        kT = singles.tile([128, T], f32r)
        v_sb = singles.tile([128, B * NKT, D + 1], f32r)  # v tiles + ones column
        yT = singles.tile([D + 1, BT], f32r)  # unnormalized y^T; row D = softmax sums
        wq_sb = singles.tile([128, NC_CH, D], bf16)
        wk_sb = singles.tile([128, NC_CH, D], bf16)
        wv_sb = singles.tile([128, NC_CH, D], bf16)
        wp_sb = singles.tile([D + 1, C], f32r)
        e_sb = singles.tile([D + 1, 2], f32r)

        nc.sync.dma_start(wq_sb[:], wq[:, :].rearrange("(a p) d -> p a d", p=128))
        nc.sync.dma_start(wk_sb[:], wk[:, :].rearrange("(a p) d -> p a d", p=128))
        nc.sync.dma_start(wv_sb[:], wv[:, :].rearrange("(a p) d -> p a d", p=128))
        nc.sync.dma_start(wp_sb[:], wp[:, :])
        nc.sync.dma_start(e_sb[:], ev[:, :])
        o = ones64[:]
        ones_bcast = bass.AP(tensor=o.tensor, offset=o.offset, ap=[[0, 128], [1, 64]])
        nc.gpsimd.dma_start(out=v_sb[:, :, D], in_=ones_bcast)

        # ---------------- Phase 1: QKV projection ----------------
        with (
            tc.tile_pool(name="p1x", bufs=4) as xpool,
            tc.tile_pool(name="p1qk", bufs=2, space="PSUM") as psqk,
            tc.tile_pool(name="p1v", bufs=4, space="PSUM") as psv,
        ):
            for j in range(NQT):
                for b in range(B):
                    t0 = b * T + j * 512
                    xt = xpool.tile([128, NC_CH, 512], bf16, tag="xt")
                    for c in range(NC_CH):
                        nc.sync.dma_start(xt[:, c, :], xTr[c, :, t0 : t0 + 512])
                    lo, hi = 64 * b, 64 * b + 64
                    tp = (0, 64) if b == 1 else None
                    pq = psqk.tile([128, 512], f32, tag="pq")
                    pk = psqk.tile([128, 512], f32, tag="pk")
                    for c in range(NC_CH):
                        nc.tensor.matmul(
                            pq[lo:hi, :],
                            lhsT=_r(wq_sb[:, c, :]),
                            rhs=_r(xt[:, c, :]),
                            start=(c == 0),
                            stop=(c == NC_CH - 1),
                            tile_position=tp,
                        )
                    for c in range(NC_CH):
                        nc.tensor.matmul(
                            pk[lo:hi, :],
                            lhsT=_r(wk_sb[:, c, :]),
                            rhs=_r(xt[:, c, :]),
                            start=(c == 0),
                            stop=(c == NC_CH - 1),
                            tile_position=tp,
                        )
                    nc.vector.tensor_copy(
                        out=qT[lo:hi, j * 512 : (j + 1) * 512], in_=pq[lo:hi, :]
                    )
                    nc.vector.tensor_copy(
                        out=kT[lo:hi, j * 512 : (j + 1) * 512], in_=pk[lo:hi, :]
                    )
                    # v in natural [T, D] layout: x-tile chunks as stationary operand
                    for rr in range(4):
                        pv = psv.tile([128, D], f32, tag="pv")
                        for c in range(NC_CH):
                            nc.tensor.matmul(
                                pv[:],
                                lhsT=_r(xt[:, c, rr * 128 : (rr + 1) * 128]),
                                rhs=_r(wv_sb[:, c, :]),
                                start=(c == 0),
                                stop=(c == NC_CH - 1),
                            )
                        rt = b * NKT + j * 4 + rr
                        nc.vector.tensor_copy(out=v_sb[:, rt, 0:D], in_=pv[:])

        # ---------------- Phase 2: causal attention ----------------
        with (
            tc.tile_pool(name="p2p", bufs=3) as ppool,
            tc.tile_pool(name="p2s", bufs=2, space="PSUM") as pss,
            tc.tile_pool(name="p2y", bufs=2, space="PSUM") as psy,
        ):
            for j in range(NQT):
                q0 = j * 512
                nkt = 4 * (j + 1)  # causal k-tiles for this q block
                groups = [
                    list(range(s, min(s + KGRP, nkt))) for s in range(0, nkt, KGRP)
                ]
                yps = [
                    psy.tile([D + 1, 512], f32, tag="y", name=f"y_{j}_{b}")
                    for b in range(B)
                ]
                for g in groups:
                    for b in range(B):
                        lo, hi = 64 * b, 64 * b + 64
                        s4 = pss.tile([128, KGRP, 512], f32, tag="s")
                        for ui, kt in enumerate(g):
                            nc.tensor.matmul(
                                s4[:, ui, :],
                                lhsT=_r(kT[lo:hi, kt * 128 : (kt + 1) * 128]),
                                rhs=_r(qT[lo:hi, q0 : q0 + 512]),
                                start=True,
                                stop=True,
                            )
                        nu = len(g)
                        p4 = ppool.tile([128, KGRP, 512], f32r, tag="p")
                        # exp(s/sqrt(D)); scores are O(1) so no max subtraction
                        nc.scalar.activation(
                            out=p4[:, 0:nu, :],
                            in_=s4[:, 0:nu, :],
                            func=mybir.ActivationFunctionType.Exp,
                            scale=0.125,
                        )
                        for ui, kt in enumerate(g):
                            dlt = kt * 128 - q0
                            if dlt > -128:
                                # keep where (q0+col) >= (kt*128+p)
                                nc.gpsimd.affine_select(
                                    out=p4[:, ui, :],
                                    in_=p4[:, ui, :],
                                    compare_op=mybir.AluOpType.is_ge,
                                    fill=0.0,
                                    base=-dlt,
                                    channel_multiplier=-1,
                                    pattern=[[1, 512]],
                                )
                        for ui, kt in enumerate(g):
                            nc.tensor.matmul(
                                yps[b][:],
                                lhsT=_r(v_sb[:, b * NKT + kt, :]),
                                rhs=_r(p4[:, ui, :]),
                                start=(kt == 0),
                                stop=(kt == nkt - 1),
                            )
                for b in range(B):
                    nc.vector.tensor_copy(
                        out=yT[:, b * T + q0 : b * T + q0 + 512], in_=yps[b][:]
                    )

        # ---------------- Phase 3: c_proj partial + normalization ----------------
        with (
            tc.tile_pool(name="p3o", bufs=3) as opool,
            tc.tile_pool(name="p3ps", bufs=2, space="PSUM") as pso,
        ):
            for r in range(BT // 128):
                lhsT = yT[:, r * 128 : (r + 1) * 128]  # [65, 128]
                po = pso.tile([128, C], f32, tag="po")
                pu = pso.tile([128, 2], f32, tag="pu")
                nc.tensor.matmul(po[:], lhsT=_r(lhsT), rhs=_r(wp_sb[:]), start=True, stop=True)
                nc.tensor.matmul(pu[:], lhsT=_r(lhsT), rhs=_r(e_sb[:]), start=True, stop=True)
                recip = opool.tile([128, 1], f32, tag="recip")
                nc.vector.reciprocal(recip[:], pu[:, 0:1])
                ot = opool.tile([128, C], f32, tag="ot")
                nc.vector.tensor_scalar_mul(ot[:], in0=po[:], scalar1=recip[:])
                nc.sync.dma_start(outp[r * 128 : (r + 1) * 128, :], ot[:])

    nc.compile()
    return nc


_cache: dict = {}


def _get_nc() -> bass.Bass:
    if "nc" not in _cache:
        _cache["nc"] = build_kernel()
    return _cache["nc"]


def make_in_maps(x, W_attn, W_proj):
    import ml_dtypes
    xTq = np.ascontiguousarray(x.reshape(BT, C).T).astype(ml_dtypes.bfloat16)
    in_maps = []
    for i in range(NCORES):
        wp_pad = np.zeros((D + 1, C), dtype=np.float32)
        wp_pad[:D] = W_proj[i * D : (i + 1) * D, :]
        ev = np.zeros((D + 1, 2), dtype=np.float32)
        ev[D, 0] = 1.0
        in_maps.append(
            {
                "xT": xTq,
                "ev": ev,
                "ones64": np.ones(64, dtype=np.float32),
                "wq": np.ascontiguousarray(W_attn[:, i * D : (i + 1) * D]).astype(ml_dtypes.bfloat16),
                "wk": np.ascontiguousarray(W_attn[:, C + i * D : C + (i + 1) * D]).astype(ml_dtypes.bfloat16),
                "wv": np.ascontiguousarray(
                    W_attn[:, 2 * C + i * D : 2 * C + (i + 1) * D]
                ).astype(ml_dtypes.bfloat16),
                "wp": wp_pad,
            }
        )
    return in_maps


def kernel(x, W_attn, W_proj, _trace=False):
    from concourse.bass_utils import run_bass_kernel_spmd

    nc = _get_nc()
    in_maps = make_in_maps(
        np.asarray(x, dtype=np.float32),
        np.asarray(W_attn, dtype=np.float32),
        np.asarray(W_proj, dtype=np.float32),
    )
    res = run_bass_kernel_spmd(
        nc, in_maps, core_ids=list(range(NCORES)), trace=_trace
    )
    out = np.zeros((BT, C), dtype=np.float32)
    for r in res.results:
        out += r["outp"]
    out = out.reshape(B, T, C)
    if _trace:
        return out, res
    return out



# revision 9
# speedup vs baseline: 1.5154x; 1.5154x over previous
"""Causal self-attention Trainium2 kernel (v2).

y = softmax_causal((x@Wq)(x@Wk)^T / sqrt(D)) @ (x@Wv) @ Wp
Sharding: head-parallel over 8 cores (H=8 heads, one per core), both batches
on every core. Each core produces its head's contribution to y @ W_proj;
the host sums the 8 partials.

v2 design notes (vs baseline):
- all matmul operands bf16 (1 cycle/row, FWL weight loads)
- x loaded via 8 large DMAs (1MB each) instead of 64 small ones
- s-matmuls for the two batches are row-tiled (partitions 0:64 / 64:128)
  and issued adjacently so the PE runs them concurrently
- softmax-sums row sits at index 0 of the v tile (ones column first) so
  the per-query reciprocal lands on partition 0 for partition_broadcast;
  y is normalized in [d, q] layout (few elements) instead of the final
  [tokens, C] layout (many elements)
- c_proj matmuls write PSUM and are DMA'd straight to HBM (no SBUF copy);
  they are emitted one q-block late to fill PE slack while the DVE/gpsimd
  normalization chain for the current block drains
- single interleaved instruction stream keeps the PE dense so the HAM
  clock gate stays at full rate
"""

import sys

sys.path.insert(0, "/opt/trn_rl_repo")

from contextlib import ExitStack

import numpy as np

import concourse.bass as bass
import concourse.mybir as mybir
import concourse.tile as tile
from concourse import bacc

B, T, C, H, D = 2, 4096, 512, 8, 64
BT = B * T  # 8192
NCORES = 8
NC_CH = C // 128  # 4 contraction chunks for the QKV projection
NJ = T // 512  # 8 q-blocks per batch
NTB = 8  # x token-blocks of 1024 (global tokens, both batches)

f32 = mybir.dt.float32
bf16 = mybir.dt.bfloat16


def build_kernel() -> bass.Bass:
    nc = bacc.Bacc()

    xT = nc.dram_tensor("xT", [C, BT], bf16, kind="ExternalInput")
    wq = nc.dram_tensor("wq", [C, D], bf16, kind="ExternalInput")
    wk = nc.dram_tensor("wk", [C, D], bf16, kind="ExternalInput")
    wv = nc.dram_tensor("wv", [C, D], bf16, kind="ExternalInput")
    # wp row 0 is zeros (multiplies the sums row of ytn); rows 1:65 are this
    # head's W_proj slice.
    wp = nc.dram_tensor("wp", [D + 1, C], bf16, kind="ExternalInput")
    outp = nc.dram_tensor("outp", [BT, C], bf16, kind="ExternalOutput")

    xTr = xT[:, :].rearrange("(a p) t -> a p t", p=128)  # [4, 128, BT]

    with tile.TileContext(nc) as tc, ExitStack() as ctx:
        singles = ctx.enter_context(tc.tile_pool(name="singles", bufs=1))

        # Persistent SBUF tensors
        qT = singles.tile([128, T], bf16)  # [0:64]=batch0 head dims, [64:128]=b1
        kT = singles.tile([128, T], bf16)
        # v tiles: col 0 = ones (softmax sums), cols 1:65 = v, col 65 pad
        v_sb = singles.tile([128, B, T // 128, D + 2], bf16)
        wq_sb = singles.tile([128, NC_CH, D], bf16)
        wk_sb = singles.tile([128, NC_CH, D], bf16)
        wv_sb = singles.tile([128, NC_CH, D], bf16)
        wp_sb = singles.tile([D + 1, C], bf16)
        xt_sb = [
            singles.tile([128, NC_CH, 1024], bf16, name=f"xt{tb}")
            for tb in range(NTB)
        ]

        nc.sync.dma_start(wq_sb[:], wq[:, :].rearrange("(a p) d -> p a d", p=128))
        nc.sync.dma_start(wk_sb[:], wk[:, :].rearrange("(a p) d -> p a d", p=128))
        nc.sync.dma_start(wv_sb[:], wv[:, :].rearrange("(a p) d -> p a d", p=128))
        nc.sync.dma_start(wp_sb[:], wp[:, :])
        nc.gpsimd.memset(v_sb[:, :, :, 0], 1.0)

        # x loads: big DMAs, ordered so batch0/batch1 blocks alternate and
        # P1 can start after the first two arrive.
        for tb in (0, 4, 1, 5, 2, 6, 3, 7):
            nc.sync.dma_start(
                xt_sb[tb][:],
                xT[:, tb * 1024 : (tb + 1) * 1024].rearrange(
                    "(a p) t -> p a t", p=128
                ),
            )

        # ---------------- Phase 1: QKV projection ----------------
        with (
            tc.tile_pool(name="p1qk", bufs=2, space="PSUM") as psqk,
            tc.tile_pool(name="p1v", bufs=2, space="PSUM") as psv,
        ):
            for j in range(NJ):
                tbs = [j // 2, 4 + j // 2]
                off = (j % 2) * 512
                pq = psqk.tile([128, 512], f32, tag="pq")
                pk = psqk.tile([128, 512], f32, tag="pk")
                for b in range(B):
                    lo = 64 * b
                    for c in range(NC_CH):
                        nc.tensor.matmul(
                            pq[lo : lo + 64, :],
                            lhsT=wq_sb[:, c, :],
                            rhs=xt_sb[tbs[b]][:, c, off : off + 512],
                            start=(c == 0),
                            stop=(c == NC_CH - 1),
                        )
                for b in range(B):
                    lo = 64 * b
                    for c in range(NC_CH):
                        nc.tensor.matmul(
                            pk[lo : lo + 64, :],
                            lhsT=wk_sb[:, c, :],
                            rhs=xt_sb[tbs[b]][:, c, off : off + 512],
                            start=(c == 0),
                            stop=(c == NC_CH - 1),
                        )
                nc.vector.tensor_copy(out=qT[:, j * 512 : (j + 1) * 512], in_=pq[:])
                nc.vector.tensor_copy(out=kT[:, j * 512 : (j + 1) * 512], in_=pk[:])
                for b in range(B):
                    pv = psv.tile([128, 4, D], f32, tag="pv")
                    for rr in range(4):
                        for c in range(NC_CH):
                            nc.tensor.matmul(
                                pv[:, rr, :],
                                lhsT=xt_sb[tbs[b]][
                                    :, c, off + rr * 128 : off + (rr + 1) * 128
                                ],
                                rhs=wv_sb[:, c, :],
                                start=(c == 0),
                                stop=(c == NC_CH - 1),
                            )
                    nc.vector.tensor_copy(
                        out=v_sb[:, b, j * 4 : j * 4 + 4, 1 : D + 1], in_=pv[:]
                    )

        # ---------------- Phase 2+3: attention + c_proj, interleaved ------
        with (
            tc.tile_pool(name="p2s", bufs=2, space="PSUM") as pss,
            tc.tile_pool(name="p2y", bufs=1, space="PSUM") as psy,
            tc.tile_pool(name="p3o", bufs=2, space="PSUM") as pso,
            tc.tile_pool(name="p2p", bufs=4) as ppool,
            tc.tile_pool(name="p2n", bufs=2) as npool,
            tc.tile_pool(name="p3s", bufs=3) as opool,
        ):
            pending = None  # (j, ytn) waiting for c_proj emission

            def emit_s(j, kt):
                q0 = j * 512
                s4 = pss.tile([128, 2, 512], f32, tag="s")
                for b in range(B):
                    lo = 64 * b
                    nc.tensor.matmul(
                        s4[:, b, :],
                        lhsT=kT[lo : lo + 64, kt * 128 : (kt + 1) * 128],
                        rhs=qT[lo : lo + 64, q0 : q0 + 512],
                        start=True,
                        stop=True,
                    )
                return s4

            def emit_po(pj, ytn):
                for b in range(B):
                    for ch in range(4):
                        po = pso.tile([128, 512], f32, tag="po")
                        nc.tensor.matmul(
                            po[:],
                            lhsT=ytn[:, b, ch * 128 : (ch + 1) * 128],
                            rhs=wp_sb[:],
                            start=True,
                            stop=True,
                        )
                        ot = opool.tile([128, C], bf16, tag="ot")
                        nc.vector.tensor_copy(out=ot[:], in_=po[:])
                        t0 = b * T + pj * 512 + ch * 128
                        nc.sync.dma_start(outp[t0 : t0 + 128, :], ot[:])

            for j in range(NJ):
                nkt = 4 * (j + 1)
                q0 = j * 512
                yps = psy.tile([D + 1, 2, 512], f32, tag="y")
                s_next = emit_s(j, 0)
                for kt in range(nkt):
                    s_cur = s_next
                    if kt + 1 < nkt:
                        s_next = emit_s(j, kt + 1)
                    if kt == 0 and pending is not None:
                        emit_po(*pending)
                        pending = None
                    p4 = ppool.tile([128, 2, 512], bf16, tag="p")
                    nc.scalar.activation(
                        out=p4[:],
                        in_=s_cur[:],
                        func=mybir.ActivationFunctionType.Exp,
                        scale=0.125,
                    )
                    if kt >= 4 * j:
                        # diagonal tile: zero non-causal entries
                        for b in range(B):
                            nc.gpsimd.affine_select(
                                out=p4[:, b, :],
                                in_=p4[:, b, :],
                                compare_op=mybir.AluOpType.is_ge,
                                fill=0.0,
                                base=q0 - kt * 128,
                                channel_multiplier=-1,
                                pattern=[[1, 512]],
                            )
                    for b in range(B):
                        nc.tensor.matmul(
                            yps[:, b, :],
                            lhsT=v_sb[:, b, kt, 0 : D + 1],
                            rhs=p4[:, b, :],
                            start=(kt == 0),
                            stop=(kt == nkt - 1),
                        )
                # normalize y in [d, q] layout: row 0 of yps holds the sums
                recip = npool.tile([1, 2, 512], f32, tag="recip")
                nc.vector.reciprocal(recip[:], yps[0:1, :, :])
                rb = npool.tile([D + 1, 2, 512], f32, tag="rb")
                nc.gpsimd.partition_broadcast(rb[:], recip[:])
                ytn = npool.tile([D + 1, 2, 512], bf16, tag="ytn")
                nc.vector.tensor_mul(ytn[:], yps[:], rb[:])
                pending = (j, ytn)
            emit_po(*pending)

    nc.compile()
    return nc


_cache: dict = {}


def _get_nc() -> bass.Bass:
    if "nc" not in _cache:
        _cache["nc"] = build_kernel()
    return _cache["nc"]


def make_in_maps(x, W_attn, W_proj):
    import ml_dtypes

    xTq = np.ascontiguousarray(x.reshape(BT, C).T).astype(ml_dtypes.bfloat16)
    in_maps = []
    for i in range(NCORES):
        wp_pad = np.zeros((D + 1, C), dtype=np.float32)
        wp_pad[1:] = W_proj[i * D : (i + 1) * D, :]
        in_maps.append(
            {
                "xT": xTq,
                "wq": np.ascontiguousarray(W_attn[:, i * D : (i + 1) * D]).astype(
                    ml_dtypes.bfloat16
                ),
                "wk": np.ascontiguousarray(
                    W_attn[:, C + i * D : C + (i + 1) * D]
                ).astype(ml_dtypes.bfloat16),
                "wv": np.ascontiguousarray(
                    W_attn[:, 2 * C + i * D : 2 * C + (i + 1) * D]
                ).astype(ml_dtypes.bfloat16),
                "wp": wp_pad.astype(ml_dtypes.bfloat16),
            }
        )
    return in_maps


def kernel(x, W_attn, W_proj, _trace=False):
    from concourse.bass_utils import run_bass_kernel_spmd

    nc = _get_nc()
    in_maps = make_in_maps(
        np.asarray(x, dtype=np.float32),
        np.asarray(W_attn, dtype=np.float32),
        np.asarray(W_proj, dtype=np.float32),
    )
    res = run_bass_kernel_spmd(
        nc, in_maps, core_ids=list(range(NCORES)), trace=_trace
    )
    out = np.zeros((BT, C), dtype=np.float32)
    for r in res.results:
        out += np.asarray(r["outp"], dtype=np.float32)
    out = out.reshape(B, T, C)
    if _trace:
        return out, res
    return out


# revision 12
# speedup vs baseline: 1.9658x; 1.2972x over previous
"""Causal self-attention Trainium2 kernel (v2).

y = softmax_causal((x@Wq)(x@Wk)^T / sqrt(D)) @ (x@Wv) @ Wp
Sharding: head-parallel over 8 cores (H=8 heads, one per core), both batches
on every core. Each core produces its head's contribution to y @ W_proj;
the host sums the 8 partials.

v2 design notes (vs baseline):
- all matmul operands bf16 (1 cycle/row, FWL weight loads)
- x loaded via 8 large DMAs (1MB each) instead of 64 small ones
- s-matmuls for the two batches are row-tiled (partitions 0:64 / 64:128)
  and issued adjacently so the PE runs them concurrently
- softmax-sums row sits at index 0 of the v tile (ones column first) so
  the per-query reciprocal lands on partition 0 for partition_broadcast;
  y is normalized in [d, q] layout (few elements) instead of the final
  [tokens, C] layout (many elements)
- c_proj matmuls write PSUM and are DMA'd straight to HBM (no SBUF copy);
  they are emitted one q-block late to fill PE slack while the DVE/gpsimd
  normalization chain for the current block drains
- single interleaved instruction stream keeps the PE dense so the HAM
  clock gate stays at full rate
"""

import sys

sys.path.insert(0, "/opt/trn_rl_repo")

from contextlib import ExitStack

import numpy as np

import concourse.bass as bass
import concourse.mybir as mybir
import concourse.tile as tile
from concourse import bacc

B, T, C, H, D = 2, 4096, 512, 8, 64
BT = B * T  # 8192
NCORES = 8
NC_CH = C // 128  # 4 contraction chunks for the QKV projection
NJ = T // 512  # 8 q-blocks per batch
NTB = 8  # x token-blocks of 1024 (global tokens, both batches)

f32 = mybir.dt.float32
bf16 = mybir.dt.bfloat16


def build_kernel() -> bass.Bass:
    nc = bacc.Bacc()

    xT = nc.dram_tensor("xT", [C, BT], bf16, kind="ExternalInput")
    wq = nc.dram_tensor("wq", [C, D], bf16, kind="ExternalInput")
    wk = nc.dram_tensor("wk", [C, D], bf16, kind="ExternalInput")
    wv = nc.dram_tensor("wv", [C, D], bf16, kind="ExternalInput")
    # wp row 0 is zeros (multiplies the sums row of ytn); rows 1:65 are this
    # head's W_proj slice.
    wp = nc.dram_tensor("wp", [D + 1, C], bf16, kind="ExternalInput")
    outp = nc.dram_tensor("outp", [BT, C], bf16, kind="ExternalOutput")

    xTr = xT[:, :].rearrange("(a p) t -> a p t", p=128)  # [4, 128, BT]

    with tile.TileContext(nc) as tc, ExitStack() as ctx:
        singles = ctx.enter_context(tc.tile_pool(name="singles", bufs=1))

        # Persistent SBUF tensors
        qT = singles.tile([128, T], bf16)  # [0:64]=batch0 head dims, [64:128]=b1
        kT = singles.tile([128, T], bf16)
        # v tiles: col 0 = ones (softmax sums), cols 1:65 = v, col 65 pad
        v_sb = singles.tile([128, B, T // 128, D + 2], bf16)
        wq_sb = singles.tile([128, NC_CH, D], bf16)
        wk_sb = singles.tile([128, NC_CH, D], bf16)
        wv_sb = singles.tile([128, NC_CH, D], bf16)
        wp_sb = singles.tile([D + 1, C], bf16)
        # e_sb picks the sums row (row 0) of ytn chunks: pu = ytn_chunk.T @ e
        e_sb = singles.tile([D + 1, 1], bf16)
        xt_sb = [
            singles.tile([128, NC_CH, 1024], bf16, name=f"xt{tb}")
            for tb in range(NTB)
        ]

        nc.sync.dma_start(wq_sb[:], wq[:, :].rearrange("(a p) d -> p a d", p=128))
        nc.sync.dma_start(wk_sb[:], wk[:, :].rearrange("(a p) d -> p a d", p=128))
        nc.sync.dma_start(wv_sb[:], wv[:, :].rearrange("(a p) d -> p a d", p=128))
        nc.sync.dma_start(wp_sb[:], wp[:, :])
        nc.gpsimd.memset(v_sb[:, :, :, 0], 1.0)
        nc.gpsimd.memset(e_sb[:, :], 0.0)
        nc.gpsimd.memset(e_sb[0:1, :], 1.0)

        # x loads: big DMAs, ordered so batch0/batch1 blocks alternate and
        # P1 can start after the first two arrive.
        for tb in (0, 4, 1, 5, 2, 6, 3, 7):
            nc.sync.dma_start(
                xt_sb[tb][:],
                xT[:, tb * 1024 : (tb + 1) * 1024].rearrange(
                    "(a p) t -> p a t", p=128
                ),
            )

        # ---------------- Phase 1: QKV projection ----------------
        with (
            tc.tile_pool(name="p1qk", bufs=2, space="PSUM") as psqk,
            tc.tile_pool(name="p1v", bufs=2, space="PSUM") as psv,
        ):
            for j in range(NJ):
                tbs = [j // 2, 4 + j // 2]
                off = (j % 2) * 512
                pq = psqk.tile([128, 512], f32, tag="pq")
                pk = psqk.tile([128, 512], f32, tag="pk")
                for b in range(B):
                    lo = 64 * b
                    for c in range(NC_CH):
                        nc.tensor.matmul(
                            pq[lo : lo + 64, :],
                            lhsT=wq_sb[:, c, :],
                            rhs=xt_sb[tbs[b]][:, c, off : off + 512],
                            start=(c == 0),
                            stop=(c == NC_CH - 1),
                        )
                for b in range(B):
                    lo = 64 * b
                    for c in range(NC_CH):
                        nc.tensor.matmul(
                            pk[lo : lo + 64, :],
                            lhsT=wk_sb[:, c, :],
                            rhs=xt_sb[tbs[b]][:, c, off : off + 512],
                            start=(c == 0),
                            stop=(c == NC_CH - 1),
                        )
                nc.vector.tensor_copy(out=qT[:, j * 512 : (j + 1) * 512], in_=pq[:])
                nc.vector.tensor_copy(out=kT[:, j * 512 : (j + 1) * 512], in_=pk[:])
                for b in range(B):
                    pv = psv.tile([128, 4, D], f32, tag="pv")
                    for rr in range(4):
                        for c in range(NC_CH):
                            nc.tensor.matmul(
                                pv[:, rr, :],
                                lhsT=xt_sb[tbs[b]][
                                    :, c, off + rr * 128 : off + (rr + 1) * 128
                                ],
                                rhs=wv_sb[:, c, :],
                                start=(c == 0),
                                stop=(c == NC_CH - 1),
                            )
                    nc.vector.tensor_copy(
                        out=v_sb[:, b, j * 4 : j * 4 + 4, 1 : D + 1], in_=pv[:]
                    )

        # ---------------- Phase 2+3: attention + c_proj, interleaved ------
        with (
            tc.tile_pool(name="p2s", bufs=2, space="PSUM") as pss,
            tc.tile_pool(name="p2y", bufs=1, space="PSUM") as psy,
            tc.tile_pool(name="p3o", bufs=1, space="PSUM") as pso,
            tc.tile_pool(name="p3u", bufs=1, space="PSUM") as psu,
            tc.tile_pool(name="p2p", bufs=4) as ppool,
            tc.tile_pool(name="p2n", bufs=2) as npool,
            tc.tile_pool(name="p3s", bufs=3) as opool,
        ):
            pending = None  # (j, ytn) waiting for c_proj emission

            def emit_s(j, kt):
                q0 = j * 512
                s4 = pss.tile([128, 2, 512], f32, tag="s")
                for b in range(B):
                    lo = 64 * b
                    nc.tensor.matmul(
                        s4[:, b, :],
                        lhsT=kT[lo : lo + 64, kt * 128 : (kt + 1) * 128],
                        rhs=qT[lo : lo + 64, q0 : q0 + 512],
                        start=True,
                        stop=True,
                    )
                return s4

            def emit_pu(ytn):
                # per-token softmax sums (row 0 of ytn) transposed into
                # [token, 1] layout via 8 tiny matmuls, then one reciprocal
                pu = psu.tile([128, 8], f32, tag="pu")
                for b in range(B):
                    for ch in range(4):
                        nc.tensor.matmul(
                            pu[:, 4 * b + ch : 4 * b + ch + 1],
                            lhsT=ytn[:, b, ch * 128 : (ch + 1) * 128],
                            rhs=e_sb[:],
                            start=True,
                            stop=True,
                        )
                recip = npool.tile([128, 8], f32, tag="recip")
                nc.vector.reciprocal(recip[:], pu[:])
                return recip

            def emit_po_chunk(pj, ytn, recip, idx):
                b, ch = idx // 4, idx % 4
                po = pso.tile([128, 512], f32, tag="po")
                nc.tensor.matmul(
                    po[:],
                    lhsT=ytn[:, b, ch * 128 : (ch + 1) * 128],
                    rhs=wp_sb[:],
                    start=True,
                    stop=True,
                )
                ot = opool.tile([128, C], bf16, tag="ot")
                nc.vector.tensor_scalar_mul(
                    ot[:], in0=po[:], scalar1=recip[:, idx : idx + 1]
                )
                t0 = b * T + pj * 512 + ch * 128
                nc.sync.dma_start(outp[t0 : t0 + 128, :], ot[:])

            for j in range(NJ):
                nkt = 4 * (j + 1)
                q0 = j * 512
                yps = psy.tile([D + 1, 2, 512], f32, tag="y")
                s_next = emit_s(j, 0)
                recip = None
                for kt in range(nkt):
                    s_cur = s_next
                    if kt + 1 < nkt:
                        s_next = emit_s(j, kt + 1)
                    if pending is not None:
                        if kt == 0:
                            recip = emit_pu(pending[1])
                        if kt < 8:
                            emit_po_chunk(pending[0], pending[1], recip, kt)
                        if kt == 7 or kt == nkt - 1:
                            pending = None
                    p4 = ppool.tile([128, 2, 512], bf16, tag="p")
                    nc.scalar.activation(
                        out=p4[:],
                        in_=s_cur[:],
                        func=mybir.ActivationFunctionType.Exp,
                        scale=0.125,
                    )
                    if kt >= 4 * j:
                        # diagonal tile: zero non-causal entries
                        for b in range(B):
                            nc.gpsimd.affine_select(
                                out=p4[:, b, :],
                                in_=p4[:, b, :],
                                compare_op=mybir.AluOpType.is_ge,
                                fill=0.0,
                                base=q0 - kt * 128,
                                channel_multiplier=-1,
                                pattern=[[1, 512]],
                            )
                    for b in range(B):
                        nc.tensor.matmul(
                            yps[:, b, :],
                            lhsT=v_sb[:, b, kt, 0 : D + 1],
                            rhs=p4[:, b, :],
                            start=(kt == 0),
                            stop=(kt == nkt - 1),
                        )
                # copy unnormalized y (+ sums row) to SBUF; normalization
                # happens per-token in the c_proj consume
                ytn = npool.tile([D + 1, 2, 512], bf16, tag="ytn")
                nc.vector.tensor_copy(out=ytn[:], in_=yps[:])
                pending = (j, ytn)
            recip = emit_pu(pending[1])
            for idx in range(8):
                emit_po_chunk(pending[0], pending[1], recip, idx)

    nc.compile()
    return nc


_cache: dict = {}


def _get_nc() -> bass.Bass:
    if "nc" not in _cache:
        _cache["nc"] = build_kernel()
    return _cache["nc"]


def make_in_maps(x, W_attn, W_proj):
    import ml_dtypes

    xTq = np.ascontiguousarray(x.reshape(BT, C).T).astype(ml_dtypes.bfloat16)
    in_maps = []
    for i in range(NCORES):
        wp_pad = np.zeros((D + 1, C), dtype=np.float32)
        wp_pad[1:] = W_proj[i * D : (i + 1) * D, :]
        in_maps.append(
            {
                "xT": xTq,
                "wq": np.ascontiguousarray(W_attn[:, i * D : (i + 1) * D]).astype(
                    ml_dtypes.bfloat16
                ),
                "wk": np.ascontiguousarray(
                    W_attn[:, C + i * D : C + (i + 1) * D]
                ).astype(ml_dtypes.bfloat16),
                "wv": np.ascontiguousarray(
                    W_attn[:, 2 * C + i * D : 2 * C + (i + 1) * D]
                ).astype(ml_dtypes.bfloat16),
                "wp": wp_pad.astype(ml_dtypes.bfloat16),
            }
        )
    return in_maps


def kernel(x, W_attn, W_proj, _trace=False):
    from concourse.bass_utils import run_bass_kernel_spmd

    nc = _get_nc()
    in_maps = make_in_maps(
        np.asarray(x, dtype=np.float32),
        np.asarray(W_attn, dtype=np.float32),
        np.asarray(W_proj, dtype=np.float32),
    )
    res = run_bass_kernel_spmd(
        nc, in_maps, core_ids=list(range(NCORES)), trace=_trace
    )
    out = np.zeros((BT, C), dtype=np.float32)
    for r in res.results:
        out += np.asarray(r["outp"], dtype=np.float32)
    out = out.reshape(B, T, C)
    if _trace:
        return out, res
    return out
